# revision 32
# baseline (speedup 1.0000x reference)
"""MoE (Mixtral-style top-2 routing, SwiGLU experts) on 8 Trainium2 cores.

Sharding: expert-parallel with on-device token dispatch. Core e holds expert
e's weights and, fully on-device:
  1. computes the gate over all T=8192 tokens (fp32 matmul),
  2. top-2 + renormalized combine weights (max8 + exp/renorm vector math),
  3. compacts the token ids routed to ITS expert (gpsimd sparse_gather),
  4. gathers those tokens' activations (gpsimd dma_gather + PE transpose),
  5. runs the SwiGLU expert on the ~2k compacted tokens in float32r
     (full-speed fp32 PE mode), folding the combine weight into the hidden,
  6. returns y^T [H, C], the compacted token ids and the routed count.
The host scatter-adds the 8 per-expert compact outputs (the unshard step).

Device layout: activations transposed ([feature, token]) so matmuls contract
over the partition axis with naturally-laid-out weights; hidden h [I, C] is
staged in DRAM between the (w1,w3) stage and the w2 stage so weights stream
once while x / h tiles stay SBUF-resident.
"""

import sys

sys.path.insert(0, "/opt/trn_rl_repo")

# The image's antenv package may lack the axon_hooks module that
# run_bass_kernel_spmd imports when tracing is requested (BASS_TRACE=1).
# Provide it (and register the real NTFF hook when available) so profiled
# runs work instead of raising ModuleNotFoundError.
try:
    import antenv.axon_hooks  # noqa: F401
except ImportError:
    try:
        import types

        import antenv

        _hooks = types.ModuleType("antenv.axon_hooks")
        _hooks._hook = None
        _hooks.set_axon_ntff_profile_hook = lambda h: setattr(_hooks, "_hook", h)
        _hooks.get_axon_ntff_profile_hook = lambda: _hooks._hook
        sys.modules["antenv.axon_hooks"] = _hooks
        antenv.axon_hooks = _hooks
        try:
            from trn_agent_boot.trn_boot import _ntff_profile_via_ctypes

            _hooks.set_axon_ntff_profile_hook(
                _ntff_profile_via_ctypes("/opt/axon/libaxon_pjrt.so"))
        except Exception:
            pass
    except Exception:
        pass

import numpy as np

import concourse.bass as bass
import concourse.mybir as mybir
from concourse import bacc
from concourse.bass_utils import run_bass_kernel_spmd
from concourse.masks import make_identity
from concourse.tile import TileContext

P = 128
T = 8192          # tokens (B*S)
H = 1024          # model dim
I = 4096          # expert hidden dim
E = 8             # experts == cores
KO = H // P       # 8  k-subtiles over H
IO = I // P       # 32 i-tiles over I
NT = 512          # matmul moving free dim (fp32 PSUM bank limit)
C = 2304          # per-expert token capacity (seed-0 max count is 2182)
CQ = None
CHUNKS = [(0, 512), (512, 512), (1024, 512), (1536, 512), (2048, 256)]
F32 = mybir.dt.float32
F32R = mybir.dt.float32r
I16 = mybir.dt.int16
U32 = mybir.dt.uint32

_NC_CACHE = {}


def _build_nc():
    from contextlib import ExitStack

    nc = bacc.Bacc(None, target_bir_lowering=False)

    x = nc.dram_tensor("x", [T, H], F32R, kind="ExternalInput")
    xT = nc.dram_tensor("xT", [H, T], F32, kind="ExternalInput")
    wg = nc.dram_tensor("wgate", [H, E], F32, kind="ExternalInput")
    w1e = nc.dram_tensor("w1e", [H, I], F32R, kind="ExternalInput")
    w3e = nc.dram_tensor("w3e", [H, I], F32R, kind="ExternalInput")
    w2e = nc.dram_tensor("w2e", [I, H], F32R, kind="ExternalInput")
    onehot = nc.dram_tensor("onehot", [P, E], F32, kind="ExternalInput")
    yTc = nc.dram_tensor("yTc", [H, C], F32, kind="ExternalOutput")
    tokc = nc.dram_tensor("tokc", [16, C // 16], F32, kind="ExternalOutput")
    nfound = nc.dram_tensor("nfound", [1, 1], U32, kind="ExternalOutput")

    xT3 = xT.rearrange("(ko p) t -> p ko t", p=P)

    with TileContext(nc) as tc:
        with (
            tc.tile_pool(name="const", bufs=1) as cpool,
            tc.tile_pool(name="dram", bufs=1, space="DRAM") as dpool,
        ):
            identity = cpool.tile([P, P], F32)
            make_identity(nc, identity[:])
            identr = cpool.tile([P, P], F32R)
            nc.vector.tensor_copy(identr[:], identity[:])
            ones = cpool.tile([P, P], F32)
            nc.gpsimd.memset(ones[:], 1.0)
            onehot_sb = cpool.tile([P, E], F32)
            nc.sync.dma_start(onehot_sb[:], onehot[:])
            wg_sb = cpool.tile([P, KO, E], F32)
            nc.sync.dma_start(wg_sb[:], wg.rearrange("(ko p) e -> p ko e", p=P))

            mid = ExitStack()     # lives through phase A' (hidden build)
            mpool = mid.enter_context(tc.tile_pool(name="mid", bufs=1))
            early = ExitStack()   # lives through compaction
            epool = early.enter_context(tc.tile_pool(name="early", bufs=1))

            logitsT = epool.tile([E, T], F32)
            lg_all = epool.tile([P, T // P, E], F32)
            m8_all = epool.tile([P, T // P, E], F32)
            g_mat = epool.tile([P, T // P], F32)
            gbc = mpool.tile([P, C], F32)          # combine weight, bcast rows
            xcT = mpool.tile([P, KO, C], F32R)     # gathered tokens, transposed
            idx128 = mpool.tile([P, C // 16], I16)
            hTc = dpool.tile([I, C], F32R)
            sc_tok = dpool.tile([P, T // P], F32)
            sc_g = dpool.tile([P, T // P], F32)

            # ---- Phase 1: gate logits^T = w_gate^T @ x -> [E, T] (fp32) ----
            with (
                tc.tile_pool(name="gx", bufs=3) as gxpool,
                tc.tile_pool(name="gps", bufs=2, space="PSUM") as gpspool,
            ):
                for tcg in range(T // NT):
                    xg = gxpool.tile([P, KO, NT], F32, tag="xg")
                    nc.sync.dma_start(xg[:], xT3[:, :, tcg * NT:(tcg + 1) * NT])
                    psg = gpspool.tile([E, NT], F32, tag="psg")
                    for ko in range(KO):
                        nc.tensor.matmul(psg[:], wg_sb[:, ko], xg[:, ko],
                                         start=(ko == 0), stop=(ko == KO - 1))
                    nc.vector.tensor_copy(logitsT[:, tcg * NT:(tcg + 1) * NT], psg[:])

            # ---- Phase 2: top-2 routing -> per-token combine weight g ----
            with (
                tc.tile_pool(name="rt", bufs=2) as rtpool,
                tc.tile_pool(name="rps", bufs=4, space="PSUM") as rpspool,
            ):
                for j in range(T // P):
                    pst = rpspool.tile([P, E], F32, tag="pst")
                    nc.tensor.transpose(pst[:], logitsT[:, j * P:(j + 1) * P],
                                        identity[:E, :E])
                    nc.vector.tensor_copy(lg_all[:, j], pst[:])
                    nc.vector.max(m8_all[:, j], lg_all[:, j])

                m1b = m8_all[:, :, 0:1]
                m2b = m8_all[:, :, 1:2]
                sub = rtpool.tile([P, T // P, E], F32)
                nc.vector.tensor_tensor(sub[:], lg_all[:],
                                        m1b.to_broadcast([P, T // P, E]),
                                        mybir.AluOpType.subtract)
                pexp = rtpool.tile([P, T // P, E], F32)
                nc.scalar.activation(pexp[:], sub[:], mybir.ActivationFunctionType.Exp)
                e2in = rtpool.tile([P, T // P], F32)
                nc.vector.tensor_tensor(e2in[:], m8_all[:, :, 1], m8_all[:, :, 0],
                                        mybir.AluOpType.subtract)
                ee = rtpool.tile([P, T // P], F32)
                nc.scalar.activation(ee[:], e2in[:], mybir.ActivationFunctionType.Exp)
                nc.vector.tensor_scalar_add(ee[:], ee[:], 1.0)
                rden = rtpool.tile([P, T // P], F32)
                nc.vector.reciprocal(rden[:], ee[:])
                ind = rtpool.tile([P, T // P, E], F32)
                nc.vector.tensor_tensor(ind[:], lg_all[:],
                                        m2b.to_broadcast([P, T // P, E]),
                                        mybir.AluOpType.is_ge)
                gall = rtpool.tile([P, T // P, E], F32)
                nc.vector.tensor_mul(gall[:], pexp[:], ind[:])
                nc.vector.tensor_mul(gall[:], gall[:],
                                     onehot_sb[:, None, :].to_broadcast([P, T // P, E]))
                nc.vector.tensor_reduce(g_mat[:], gall[:], axis=mybir.AxisListType.X,
                                        op=mybir.AluOpType.add)
                nc.vector.tensor_mul(g_mat[:], g_mat[:], rden[:])

                # ---- Phase 2b: compact this expert's token list ----
                indsel = rtpool.tile([P, T // P], F32)
                nc.vector.tensor_scalar(indsel[:], g_mat[:], 0.0, None,
                                        mybir.AluOpType.not_equal)
                sel1 = rtpool.tile([P, T // P], F32)
                nc.vector.tensor_scalar_add(sel1[:], indsel[:], -1.0)
                gv = rtpool.tile([P, T // P], F32)
                nc.vector.tensor_add(gv[:], g_mat[:], sel1[:])
                tokp1 = rtpool.tile([P, T // P], F32)
                nc.gpsimd.iota(tokp1[:], pattern=[[P, T // P]], base=1,
                               channel_multiplier=1,
                               allow_small_or_imprecise_dtypes=True)
                tokv = rtpool.tile([P, T // P], F32)
                nc.vector.tensor_mul(tokv[:], tokp1[:], indsel[:])
                nc.vector.tensor_scalar_add(tokv[:], tokv[:], -1.0)
                nc.sync.dma_start(sc_tok[:], tokv[:])
                nc.sync.dma_start(sc_g[:], gv[:])
                tok16 = rtpool.tile([16, T // 16], F32)
                nc.sync.dma_start(tok16[:],
                                  sc_tok[:].rearrange("(a r) j -> a (r j)", a=16))
                g16 = rtpool.tile([16, T // 16], F32)
                nc.sync.dma_start(g16[:],
                                  sc_g[:].rearrange("(a r) j -> a (r j)", a=16))
                tokc16 = rtpool.tile([16, C // 16], F32)
                nf = rtpool.tile([1, 1], U32)
                nc.gpsimd.sparse_gather(tokc16[:], tok16[:], num_found=nf[:])
                gc16 = rtpool.tile([16, C // 16], F32)
                nf2 = rtpool.tile([1, 1], U32)
                nc.gpsimd.sparse_gather(gc16[:], g16[:], num_found=nf2[:])
                nc.sync.dma_start(tokc[:], tokc16[:])
                nc.sync.dma_start(nfound[:], nf[:])

                tokcl = rtpool.tile([16, C // 16], F32)
                nc.vector.tensor_scalar(tokcl[:], tokc16[:], 0.0, float(T - 1),
                                        mybir.AluOpType.max, mybir.AluOpType.min)
                idx16i = rtpool.tile([16, C // 16], I16)
                nc.vector.tensor_copy(idx16i[:], tokcl[:])
                for k in range(8):
                    nc.sync.dma_start(idx128[16 * k:16 * (k + 1), :], idx16i[:])

                # ---- Phase 2c: broadcast g over partitions -> gbc [P, C] ----
                # per 512-slot chunk: interleave-expand g (slot k lives at
                # [k%16, k//16]) then ones^T @ masked -> every row = g
                for co, cw in CHUNKS:
                    rhsx = rtpool.tile([16, NT // 16, 16], F32, tag="rhsx")
                    nc.gpsimd.affine_select(
                        out=rhsx[:, :cw // 16],
                        in_=gc16[:, co // 16:(co + cw) // 16, None]
                        .to_broadcast([16, cw // 16, 16]),
                        compare_op=mybir.AluOpType.is_equal,
                        fill=0.0,
                        base=0,
                        pattern=[[0, cw // 16], [1, 16]],
                        channel_multiplier=-1,
                    )
                    psb = rpspool.tile([P, NT], F32, tag="psb")
                    nc.tensor.matmul(psb[:, :cw], ones[:16, :],
                                     rhsx[:, :cw // 16].rearrange("p a b -> p (a b)"),
                                     start=True, stop=True)
                    nc.vector.tensor_copy(gbc[:, co:co + cw], psb[:, :cw])

            early.close()

            # ---- Phase 2d: gather routed tokens, transpose to [H, C] ----
            with (
                tc.tile_pool(name="gr", bufs=2) as grpool,
                tc.tile_pool(name="grps", bufs=4, space="PSUM") as grpspool,
            ):
                for co, cw in CHUNKS:
                    xrq = grpool.tile([P, NT // P, H], F32R, tag="xrq")
                    nc.gpsimd.dma_gather(
                        xrq[:, :cw // P], x[:],
                        idx128[:, co // 16:(co + cw) // 16],
                        num_idxs=cw, num_idxs_reg=cw, elem_size=H,
                        transpose=False, queue_num=0)
                    for c4 in range(cw // P):
                        for h8 in range(KO):
                            pst2 = grpspool.tile([P, P], F32R, tag="pst2")
                            nc.tensor.transpose(
                                pst2[:], xrq[:, c4, h8 * P:(h8 + 1) * P], identr[:])
                            off = co + c4 * P
                            nc.vector.tensor_copy(xcT[:, h8, off:off + P], pst2[:])

            # ---- Phase A': h^T = silu(w1^T xc) * (w3^T xc) * g -> DRAM ----
            with (
                tc.tile_pool(name="aw", bufs=2) as awpool,
                tc.tile_pool(name="ah", bufs=3) as ahpool,
                tc.tile_pool(name="aps", bufs=2, space="PSUM") as apspool,
            ):
                for i in range(IO):
                    w1s = awpool.tile([P, KO, P], F32R, tag="w1s")
                    nc.sync.dma_start(
                        w1s[:], w1e[:, i * P:(i + 1) * P]
                        .rearrange("(ko p) q -> p ko q", p=P))
                    w3s = awpool.tile([P, KO, P], F32R, tag="w3s")
                    nc.sync.dma_start(
                        w3s[:], w3e[:, i * P:(i + 1) * P]
                        .rearrange("(ko p) q -> p ko q", p=P))
                    for co, cw in CHUNKS:
                        ps1 = apspool.tile([P, NT], F32, tag="ps1")
                        for ko in range(KO):
                            nc.tensor.matmul(ps1[:, :cw], w1s[:, ko],
                                             xcT[:, ko, co:co + cw],
                                             start=(ko == 0), stop=(ko == KO - 1))
                        ps3 = apspool.tile([P, NT], F32, tag="ps3")
                        for ko in range(KO):
                            nc.tensor.matmul(ps3[:, :cw], w3s[:, ko],
                                             xcT[:, ko, co:co + cw],
                                             start=(ko == 0), stop=(ko == KO - 1))
                        hsil = ahpool.tile([P, NT], F32R, tag="hsil")
                        nc.scalar.activation(hsil[:, :cw], ps1[:, :cw],
                                             mybir.ActivationFunctionType.Silu)
                        h3g = ahpool.tile([P, NT], F32R, tag="h3g")
                        nc.vector.tensor_mul(h3g[:, :cw], ps3[:, :cw],
                                             gbc[:, co:co + cw])
                        htile = ahpool.tile([P, NT], F32R, tag="htile")
                        nc.vector.tensor_mul(htile[:, :cw], hsil[:, :cw],
                                             h3g[:, :cw])
                        nc.sync.dma_start(
                            hTc[i * P:(i + 1) * P, co:co + cw], htile[:, :cw])

            mid.close()

            # ---- Phase B': y^T = w2^T @ h^T -> [H, C] ----
            hT4 = hTc[:].rearrange("(io p) t -> p io t", p=P)
            with (
                tc.tile_pool(name="bw", bufs=1) as bwpool,
                tc.tile_pool(name="bh", bufs=2) as bhpool,
                tc.tile_pool(name="by", bufs=3) as bypool,
                tc.tile_pool(name="bps", bufs=1, space="PSUM") as bpspool,
            ):
                w2sb = bwpool.tile([P, IO, H], F32R)
                w2r = w2e.rearrange("(io p) h -> p io h", p=P)
                QI = 8
                for qw in range(IO // QI):
                    nc.sync.dma_start(w2sb[:, qw * QI:(qw + 1) * QI, :],
                                      w2r[:, qw * QI:(qw + 1) * QI, :])
                for tcy, (co, cw) in enumerate(CHUNKS):
                    psy = [bpspool.tile([P, NT], F32, tag=f"psy{m}",
                                        name=f"psy{m}_{tcy}")
                           for m in range(H // P)]
                    for qg in range(IO // QI):
                        hq = bhpool.tile([P, QI, NT], F32R, tag="hq")
                        nc.sync.dma_start(
                            hq[:, :, :cw],
                            hT4[:, qg * QI:(qg + 1) * QI, co:co + cw])
                        for m in range(H // P):
                            for i8 in range(QI):
                                io = qg * QI + i8
                                nc.tensor.matmul(
                                    psy[m][:, :cw],
                                    w2sb[:, io, m * P:(m + 1) * P],
                                    hq[:, i8, :cw],
                                    start=(io == 0), stop=(io == IO - 1))
                    for m in range(H // P):
                        yt = bypool.tile([P, NT], F32, tag="yt")
                        nc.vector.tensor_copy(yt[:, :cw], psy[m][:, :cw])
                        nc.sync.dma_start(
                            yTc[m * P:(m + 1) * P, co:co + cw], yt[:, :cw])

    nc.finalize()
    return nc


def _get_nc():
    if "nc" not in _NC_CACHE:
        _NC_CACHE["nc"] = _build_nc()
    return _NC_CACHE["nc"]


def kernel(x, w_gate, w1, w2, w3, num_experts_per_tok):
    assert int(num_experts_per_tok) == 2
    B, S, _H = x.shape
    assert (B * S, _H) == (T, H)

    xf = np.ascontiguousarray(np.asarray(x, dtype=np.float32).reshape(T, H))
    xTh = np.ascontiguousarray(xf.T)
    wgh = np.ascontiguousarray(np.asarray(w_gate, dtype=np.float32))
    w1h = np.asarray(w1, dtype=np.float32)
    w2h = np.asarray(w2, dtype=np.float32)
    w3h = np.asarray(w3, dtype=np.float32)

    in_maps = []
    for e in range(E):
        oh = np.zeros((P, E), dtype=np.float32)
        oh[:, e] = 1.0
        in_maps.append({
            "x": xf,
            "xT": xTh,
            "wgate": wgh,
            "w1e": np.ascontiguousarray(w1h[e]),
            "w3e": np.ascontiguousarray(w3h[e]),
            "w2e": np.ascontiguousarray(w2h[e]),
            "onehot": oh,
        })

    nc = _get_nc()
    res = run_bass_kernel_spmd(nc, in_maps, core_ids=list(range(E)))
    global LAST_EXEC_NS
    LAST_EXEC_NS = res.exec_time_ns

    acc = np.zeros((T, H), dtype=np.float32)
    for r in res.results:
        n = int(r["nfound"][0, 0])
        assert n <= C, f"capacity overflow: {n} > {C}"
        tok = np.rint(r["tokc"].T.ravel()[:n]).astype(np.int64)
        assert tok.min() >= 0 and tok.max() < T
        assert len(np.unique(tok)) == n
        acc[tok] += r["yTc"].T[:n]
    return acc.reshape(B, S, H).astype(np.float32)


# revision 33
# speedup vs baseline: 1.0162x; 1.0162x over previous
"""MoE (Mixtral-style top-2 routing, SwiGLU experts) on 8 Trainium2 cores.

Sharding: expert-parallel with on-device token dispatch. Core e holds expert
e's weights and, fully on-device:
  1. computes the gate over all T=8192 tokens (fp32 matmul),
  2. top-2 + renormalized combine weights (max8 + exp/renorm vector math),
  3. compacts the token ids routed to ITS expert (gpsimd sparse_gather),
  4. gathers those tokens' activations (gpsimd dma_gather + PE transpose),
  5. runs the SwiGLU expert on the ~2k compacted tokens in float32r
     (full-speed fp32 PE mode), folding the combine weight into the hidden,
  6. returns y^T [H, C], the compacted token ids and the routed count.
The host scatter-adds the 8 per-expert compact outputs (the unshard step).

Device layout: activations transposed ([feature, token]) so matmuls contract
over the partition axis with naturally-laid-out weights; hidden h [I, C] is
staged in DRAM between the (w1,w3) stage and the w2 stage so weights stream
once while x / h tiles stay SBUF-resident.
"""

import sys

sys.path.insert(0, "/opt/trn_rl_repo")

# The image's antenv package may lack the axon_hooks module that
# run_bass_kernel_spmd imports when tracing is requested (BASS_TRACE=1).
# Provide it (and register the real NTFF hook when available) so profiled
# runs work instead of raising ModuleNotFoundError.
try:
    import antenv.axon_hooks  # noqa: F401
except ImportError:
    try:
        import types

        import antenv

        _hooks = types.ModuleType("antenv.axon_hooks")
        _hooks._hook = None
        _hooks.set_axon_ntff_profile_hook = lambda h: setattr(_hooks, "_hook", h)
        _hooks.get_axon_ntff_profile_hook = lambda: _hooks._hook
        sys.modules["antenv.axon_hooks"] = _hooks
        antenv.axon_hooks = _hooks
        try:
            from trn_agent_boot.trn_boot import _ntff_profile_via_ctypes

            _hooks.set_axon_ntff_profile_hook(
                _ntff_profile_via_ctypes("/opt/axon/libaxon_pjrt.so"))
        except Exception:
            pass
    except Exception:
        pass

import numpy as np

import concourse.bass as bass
import concourse.mybir as mybir
from concourse import bacc
from concourse.bass_utils import run_bass_kernel_spmd
from concourse.masks import make_identity
from concourse.tile import TileContext

P = 128
T = 8192          # tokens (B*S)
H = 1024          # model dim
I = 4096          # expert hidden dim
E = 8             # experts == cores
KO = H // P       # 8  k-subtiles over H
IO = I // P       # 32 i-tiles over I
NT = 512          # matmul moving free dim (fp32 PSUM bank limit)
C = 2304          # per-expert token capacity (seed-0 max count is 2182)
CQ = None
CHUNKS = [(0, 512), (512, 512), (1024, 512), (1536, 512), (2048, 256)]
F32 = mybir.dt.float32
F32R = mybir.dt.float32r
I16 = mybir.dt.int16
U32 = mybir.dt.uint32

_NC_CACHE = {}


def _build_nc():
    from contextlib import ExitStack

    nc = bacc.Bacc(None, target_bir_lowering=False)

    x = nc.dram_tensor("x", [T, H], F32R, kind="ExternalInput")
    xT = nc.dram_tensor("xT", [H, T], F32, kind="ExternalInput")
    wg = nc.dram_tensor("wgate", [H, E], F32, kind="ExternalInput")
    w1e = nc.dram_tensor("w1e", [H, I], F32R, kind="ExternalInput")
    w3e = nc.dram_tensor("w3e", [H, I], F32R, kind="ExternalInput")
    w2e = nc.dram_tensor("w2e", [I, H], F32R, kind="ExternalInput")
    onehot = nc.dram_tensor("onehot", [P, E], F32, kind="ExternalInput")
    yTc = nc.dram_tensor("yTc", [H, C], F32, kind="ExternalOutput")
    tokc = nc.dram_tensor("tokc", [16, C // 16], F32, kind="ExternalOutput")
    nfound = nc.dram_tensor("nfound", [1, 1], U32, kind="ExternalOutput")

    xT3 = xT.rearrange("(ko p) t -> p ko t", p=P)

    with TileContext(nc) as tc:
        with (
            tc.tile_pool(name="const", bufs=1) as cpool,
            tc.tile_pool(name="dram", bufs=1, space="DRAM") as dpool,
        ):
            identity = cpool.tile([P, P], F32)
            make_identity(nc, identity[:])
            identr = cpool.tile([P, P], F32R)
            nc.vector.tensor_copy(identr[:], identity[:])
            ones = cpool.tile([P, P], F32)
            nc.gpsimd.memset(ones[:], 1.0)
            onehot_sb = cpool.tile([P, E], F32)
            nc.sync.dma_start(onehot_sb[:], onehot[:])
            wg_sb = cpool.tile([P, KO, E], F32)
            nc.sync.dma_start(wg_sb[:], wg.rearrange("(ko p) e -> p ko e", p=P))

            mid = ExitStack()     # lives through phase A' (hidden build)
            mpool = mid.enter_context(tc.tile_pool(name="mid", bufs=1))
            early = ExitStack()   # lives through compaction
            epool = early.enter_context(tc.tile_pool(name="early", bufs=1))

            logitsT = epool.tile([E, T], F32)
            lg_all = epool.tile([P, T // P, E], F32)
            m8_all = epool.tile([P, T // P, E], F32)
            g_mat = epool.tile([P, T // P], F32)
            gbc = mpool.tile([P, C], F32)          # combine weight, bcast rows
            xcT = mpool.tile([P, KO, C], F32R)     # gathered tokens, transposed
            idx128 = mpool.tile([P, C // 16], I16)
            hTc = dpool.tile([I, C], F32R)
            sc_tok = dpool.tile([P, T // P], F32)
            sc_g = dpool.tile([P, T // P], F32)

            # ---- Phase 1: gate logits^T = w_gate^T @ x -> [E, T] (fp32) ----
            with (
                tc.tile_pool(name="gx", bufs=3) as gxpool,
                tc.tile_pool(name="gps", bufs=2, space="PSUM") as gpspool,
            ):
                for tcg in range(T // NT):
                    xg = gxpool.tile([P, KO, NT], F32, tag="xg")
                    nc.sync.dma_start(xg[:], xT3[:, :, tcg * NT:(tcg + 1) * NT])
                    psg = gpspool.tile([E, NT], F32, tag="psg")
                    for ko in range(KO):
                        nc.tensor.matmul(psg[:], wg_sb[:, ko], xg[:, ko],
                                         start=(ko == 0), stop=(ko == KO - 1))
                    nc.vector.tensor_copy(logitsT[:, tcg * NT:(tcg + 1) * NT], psg[:])

            # ---- Phase 2: top-2 routing -> per-token combine weight g ----
            with (
                tc.tile_pool(name="rt", bufs=2) as rtpool,
                tc.tile_pool(name="rps", bufs=4, space="PSUM") as rpspool,
            ):
                for j in range(T // P):
                    pst = rpspool.tile([P, E], F32, tag="pst")
                    nc.tensor.transpose(pst[:], logitsT[:, j * P:(j + 1) * P],
                                        identity[:E, :E])
                    nc.vector.tensor_copy(lg_all[:, j], pst[:])
                    nc.vector.max(m8_all[:, j], lg_all[:, j])

                m1b = m8_all[:, :, 0:1]
                m2b = m8_all[:, :, 1:2]
                sub = rtpool.tile([P, T // P, E], F32)
                nc.vector.tensor_tensor(sub[:], lg_all[:],
                                        m1b.to_broadcast([P, T // P, E]),
                                        mybir.AluOpType.subtract)
                pexp = rtpool.tile([P, T // P, E], F32)
                nc.scalar.activation(pexp[:], sub[:], mybir.ActivationFunctionType.Exp)
                e2in = rtpool.tile([P, T // P], F32)
                nc.vector.tensor_tensor(e2in[:], m8_all[:, :, 1], m8_all[:, :, 0],
                                        mybir.AluOpType.subtract)
                ee = rtpool.tile([P, T // P], F32)
                nc.scalar.activation(ee[:], e2in[:], mybir.ActivationFunctionType.Exp)
                nc.vector.tensor_scalar_add(ee[:], ee[:], 1.0)
                rden = rtpool.tile([P, T // P], F32)
                nc.vector.reciprocal(rden[:], ee[:])
                ind = rtpool.tile([P, T // P, E], F32)
                nc.vector.tensor_tensor(ind[:], lg_all[:],
                                        m2b.to_broadcast([P, T // P, E]),
                                        mybir.AluOpType.is_ge)
                gall = rtpool.tile([P, T // P, E], F32)
                nc.vector.tensor_mul(gall[:], pexp[:], ind[:])
                nc.vector.tensor_mul(gall[:], gall[:],
                                     onehot_sb[:, None, :].to_broadcast([P, T // P, E]))
                nc.vector.tensor_reduce(g_mat[:], gall[:], axis=mybir.AxisListType.X,
                                        op=mybir.AluOpType.add)
                nc.vector.tensor_mul(g_mat[:], g_mat[:], rden[:])

                # ---- Phase 2b: compact this expert's token list ----
                indsel = rtpool.tile([P, T // P], F32)
                nc.vector.tensor_scalar(indsel[:], g_mat[:], 0.0, None,
                                        mybir.AluOpType.not_equal)
                sel1 = rtpool.tile([P, T // P], F32)
                nc.vector.tensor_scalar_add(sel1[:], indsel[:], -1.0)
                gv = rtpool.tile([P, T // P], F32)
                nc.vector.tensor_add(gv[:], g_mat[:], sel1[:])
                tokp1 = rtpool.tile([P, T // P], F32)
                nc.gpsimd.iota(tokp1[:], pattern=[[P, T // P]], base=1,
                               channel_multiplier=1,
                               allow_small_or_imprecise_dtypes=True)
                tokv = rtpool.tile([P, T // P], F32)
                nc.vector.tensor_mul(tokv[:], tokp1[:], indsel[:])
                nc.vector.tensor_scalar_add(tokv[:], tokv[:], -1.0)
                nc.sync.dma_start(sc_tok[:], tokv[:])
                nc.sync.dma_start(sc_g[:], gv[:])
                tok16 = rtpool.tile([16, T // 16], F32)
                nc.sync.dma_start(tok16[:],
                                  sc_tok[:].rearrange("(a r) j -> a (r j)", a=16))
                g16 = rtpool.tile([16, T // 16], F32)
                nc.sync.dma_start(g16[:],
                                  sc_g[:].rearrange("(a r) j -> a (r j)", a=16))
                tokc16 = rtpool.tile([16, C // 16], F32)
                nf = rtpool.tile([1, 1], U32)
                nc.gpsimd.sparse_gather(tokc16[:], tok16[:], num_found=nf[:])
                gc16 = rtpool.tile([16, C // 16], F32)
                nf2 = rtpool.tile([1, 1], U32)
                nc.gpsimd.sparse_gather(gc16[:], g16[:], num_found=nf2[:])
                nc.sync.dma_start(tokc[:], tokc16[:])
                nc.sync.dma_start(nfound[:], nf[:])

                tokcl = rtpool.tile([16, C // 16], F32)
                nc.vector.tensor_scalar(tokcl[:], tokc16[:], 0.0, float(T - 1),
                                        mybir.AluOpType.max, mybir.AluOpType.min)
                idx16i = rtpool.tile([16, C // 16], I16)
                nc.vector.tensor_copy(idx16i[:], tokcl[:])
                for k in range(8):
                    nc.sync.dma_start(idx128[16 * k:16 * (k + 1), :], idx16i[:])

                # ---- Phase 2c: broadcast g over partitions -> gbc [P, C] ----
                # per 512-slot chunk: interleave-expand g (slot k lives at
                # [k%16, k//16]) then ones^T @ masked -> every row = g
                for co, cw in CHUNKS:
                    rhsx = rtpool.tile([16, NT // 16, 16], F32, tag="rhsx")
                    nc.gpsimd.affine_select(
                        out=rhsx[:, :cw // 16],
                        in_=gc16[:, co // 16:(co + cw) // 16, None]
                        .to_broadcast([16, cw // 16, 16]),
                        compare_op=mybir.AluOpType.is_equal,
                        fill=0.0,
                        base=0,
                        pattern=[[0, cw // 16], [1, 16]],
                        channel_multiplier=-1,
                    )
                    psb = rpspool.tile([P, NT], F32, tag="psb")
                    nc.tensor.matmul(psb[:, :cw], ones[:16, :],
                                     rhsx[:, :cw // 16].rearrange("p a b -> p (a b)"),
                                     start=True, stop=True)
                    nc.vector.tensor_copy(gbc[:, co:co + cw], psb[:, :cw])

            early.close()

            # ---- Phase 2d: gather routed tokens, transpose to [H, C] ----
            with (
                tc.tile_pool(name="gr", bufs=2) as grpool,
                tc.tile_pool(name="grps", bufs=4, space="PSUM") as grpspool,
            ):
                for co, cw in CHUNKS:
                    xrq = grpool.tile([P, NT // P, H], F32R, tag="xrq")
                    nc.gpsimd.dma_gather(
                        xrq[:, :cw // P], x[:],
                        idx128[:, co // 16:(co + cw) // 16],
                        num_idxs=cw, num_idxs_reg=cw, elem_size=H,
                        transpose=False, queue_num=0)
                    for c4 in range(cw // P):
                        for h8 in range(KO):
                            pst2 = grpspool.tile([P, P], F32R, tag="pst2")
                            nc.tensor.transpose(
                                pst2[:], xrq[:, c4, h8 * P:(h8 + 1) * P], identr[:])
                            off = co + c4 * P
                            nc.vector.tensor_copy(xcT[:, h8, off:off + P], pst2[:])

            # ---- Phase A': h^T = silu(w1^T xc) * (w3^T xc) * g -> DRAM ----
            with (
                tc.tile_pool(name="aw", bufs=2) as awpool,
                tc.tile_pool(name="ah", bufs=3) as ahpool,
                tc.tile_pool(name="aps", bufs=2, space="PSUM") as apspool,
            ):
                for i in range(IO):
                    w1s = awpool.tile([P, KO, P], F32R, tag="w1s")
                    nc.sync.dma_start(
                        w1s[:], w1e[:, i * P:(i + 1) * P]
                        .rearrange("(ko p) q -> p ko q", p=P))
                    w3s = awpool.tile([P, KO, P], F32R, tag="w3s")
                    nc.sync.dma_start(
                        w3s[:], w3e[:, i * P:(i + 1) * P]
                        .rearrange("(ko p) q -> p ko q", p=P))
                    for co, cw in CHUNKS:
                        ps1 = apspool.tile([P, NT], F32, tag="ps1")
                        for ko in range(KO):
                            nc.tensor.matmul(ps1[:, :cw], w1s[:, ko],
                                             xcT[:, ko, co:co + cw],
                                             start=(ko == 0), stop=(ko == KO - 1))
                        ps3 = apspool.tile([P, NT], F32, tag="ps3")
                        for ko in range(KO):
                            nc.tensor.matmul(ps3[:, :cw], w3s[:, ko],
                                             xcT[:, ko, co:co + cw],
                                             start=(ko == 0), stop=(ko == KO - 1))
                        hsil = ahpool.tile([P, NT], F32R, tag="hsil")
                        nc.scalar.activation(hsil[:, :cw], ps1[:, :cw],
                                             mybir.ActivationFunctionType.Silu)
                        h3g = ahpool.tile([P, NT], F32R, tag="h3g")
                        nc.vector.tensor_mul(h3g[:, :cw], ps3[:, :cw],
                                             gbc[:, co:co + cw])
                        htile = ahpool.tile([P, NT], F32R, tag="htile")
                        nc.vector.tensor_mul(htile[:, :cw], hsil[:, :cw],
                                             h3g[:, :cw])
                        nc.sync.dma_start(
                            hTc[i * P:(i + 1) * P, co:co + cw], htile[:, :cw])

            mid.close()

            # ---- Phase B': y^T = w2^T @ h^T -> [H, C] ----
            hT4 = hTc[:].rearrange("(io p) t -> p io t", p=P)
            with (
                tc.tile_pool(name="bw", bufs=1) as bwpool,
                tc.tile_pool(name="bh", bufs=3) as bhpool,
                tc.tile_pool(name="by", bufs=3) as bypool,
                tc.tile_pool(name="bps", bufs=1, space="PSUM") as bpspool,
            ):
                w2sb = bwpool.tile([P, IO, H], F32R)
                w2r = w2e.rearrange("(io p) h -> p io h", p=P)
                QI = 8
                for qw in range(IO // 4):
                    nc.sync.dma_start(w2sb[:, qw * 4:(qw + 1) * 4, :],
                                      w2r[:, qw * 4:(qw + 1) * 4, :])
                for tcy, (co, cw) in enumerate(CHUNKS):
                    psy = [bpspool.tile([P, NT], F32, tag=f"psy{m}",
                                        name=f"psy{m}_{tcy}")
                           for m in range(H // P)]
                    for qg in range(IO // QI):
                        hq = bhpool.tile([P, QI, NT], F32R, tag="hq")
                        nc.sync.dma_start(
                            hq[:, :, :cw],
                            hT4[:, qg * QI:(qg + 1) * QI, co:co + cw])
                        for m in range(H // P):
                            for i8 in range(QI):
                                io = qg * QI + i8
                                nc.tensor.matmul(
                                    psy[m][:, :cw],
                                    w2sb[:, io, m * P:(m + 1) * P],
                                    hq[:, i8, :cw],
                                    start=(io == 0), stop=(io == IO - 1))
                    for m in range(H // P):
                        yt = bypool.tile([P, NT], F32, tag="yt")
                        nc.vector.tensor_copy(yt[:, :cw], psy[m][:, :cw])
                        nc.sync.dma_start(
                            yTc[m * P:(m + 1) * P, co:co + cw], yt[:, :cw])

    nc.finalize()
    return nc


def _get_nc():
    if "nc" not in _NC_CACHE:
        _NC_CACHE["nc"] = _build_nc()
    return _NC_CACHE["nc"]


def kernel(x, w_gate, w1, w2, w3, num_experts_per_tok):
    assert int(num_experts_per_tok) == 2
    B, S, _H = x.shape
    assert (B * S, _H) == (T, H)

    xf = np.ascontiguousarray(np.asarray(x, dtype=np.float32).reshape(T, H))
    xTh = np.ascontiguousarray(xf.T)
    wgh = np.ascontiguousarray(np.asarray(w_gate, dtype=np.float32))
    w1h = np.asarray(w1, dtype=np.float32)
    w2h = np.asarray(w2, dtype=np.float32)
    w3h = np.asarray(w3, dtype=np.float32)

    in_maps = []
    for e in range(E):
        oh = np.zeros((P, E), dtype=np.float32)
        oh[:, e] = 1.0
        in_maps.append({
            "x": xf,
            "xT": xTh,
            "wgate": wgh,
            "w1e": np.ascontiguousarray(w1h[e]),
            "w3e": np.ascontiguousarray(w3h[e]),
            "w2e": np.ascontiguousarray(w2h[e]),
            "onehot": oh,
        })

    nc = _get_nc()
    res = run_bass_kernel_spmd(nc, in_maps, core_ids=list(range(E)))
    global LAST_EXEC_NS
    LAST_EXEC_NS = res.exec_time_ns

    acc = np.zeros((T, H), dtype=np.float32)
    for r in res.results:
        n = int(r["nfound"][0, 0])
        assert n <= C, f"capacity overflow: {n} > {C}"
        tok = np.rint(r["tokc"].T.ravel()[:n]).astype(np.int64)
        assert tok.min() >= 0 and tok.max() < T
        assert len(np.unique(tok)) == n
        acc[tok] += r["yTc"].T[:n]
    return acc.reshape(B, S, H).astype(np.float32)
